# revision 1
# baseline (speedup 1.0000x reference)
"""ContraCLM token-level contrastive loss on 8 Trainium2 NeuronCores.

Data-parallel over the batch: core b handles sample b (B=8). Per core,
with S=1536, D=1024, T=0.05:

  f_v = l2norm(h_v) with masked token rows zeroed (mask folded into the
  rsqrt scale);  F = [f1; f2]  (2S x D, bf16, stored transposed as [D, 2S])

  sim = F F^T computed as 24 x 6 grid of [128, 512] PSUM strips (K=1024).
  exp(sim/T) row sums come free from the ScalarE activation free-dim
  accumulator. Diagonal-block strips (self-sim and positive-counterpart
  entries, which land on 128-block diagonals because 2S is a multiple of
  128 and partner offset is S) get the diagonal zeroed via affine_select
  before a DVE row-sum instead.

  Masked columns were zeroed in F, so each masked column contributes
  exp(0)=1 to a row sum: subtract K0 = 2S - 2n afterwards.
  pos_sim is computed exactly in fp32 as a row-wise dot product.
  per_tok = log(Ng + exp(pos_sim/T)) - pos_sim/T; masked mean over 2n
  tokens; AllReduce-mean across the 8 cores.
"""

import sys

for _p in ("/opt/trn_rl_repo", "/opt/pypackages"):
    if _p not in sys.path:
        sys.path.append(_p)

from contextlib import ExitStack

import numpy as np

import bass_rust

import concourse.bass as bass
import concourse.tile as tile
from concourse import mybir
from concourse.bass_utils import run_bass_kernel_spmd
from concourse.masks import make_identity
from concourse.vector_clock import ScopedClock

# The walrus build in this container encodes at most 2 sync waits per
# instruction (bass_rust's inst_waits_full agrees), but Tile's semaphore
# assignment can attach more. Hoist excess waits onto unfusable same-engine
# NoOps immediately before the instruction — the engine executes its queue
# in order, so semantics are preserved.
_MAX_WAITS = 1


def _split_excess_waits(nc, ordered):
    for bb_name, insts in ordered.items():
        out = []
        changed = False
        for inst in insts:
            si = getattr(inst, "sync_info", None)
            waits = list(si.on_wait) if si is not None else []
            if len(waits) > _MAX_WAITS:
                changed = True
                extra, keep = waits[:-_MAX_WAITS], waits[-_MAX_WAITS:]
                for i in range(0, len(extra), _MAX_WAITS):
                    out.append(mybir.InstNoOp(
                        name=nc.get_next_instruction_name(),
                        sync_info=mybir.SyncInfo(
                            on_wait=extra[i:i + _MAX_WAITS], on_update=[]),
                        bass_nofuse=True,
                        engine=inst.engine,
                    ))
                si.on_wait = keep
            out.append(inst)
        if changed:
            insts[:] = out


_orig_lower_ordered_insts = tile.TileContext._lower_ordered_insts


def _patched_lower_ordered_insts(self, ordered):
    _split_excess_waits(self.nc, ordered)
    return _orig_lower_ordered_insts(self, ordered)


tile.TileContext._lower_ordered_insts = _patched_lower_ordered_insts


def _split_waits_drain_and_barrier(self, tick_clock, wait_clock):
    nc = self.nc
    probe = nc.sync.nop(nofuse=True)
    wait_clock.add_sem_waits(
        probe.ins, ScopedClock({None: tick_clock.global_clock}))
    si = probe.ins.sync_info
    waits = list(si.on_wait) if si is not None else []
    if len(waits) > _MAX_WAITS:
        si.on_wait = waits[:_MAX_WAITS]
        for i in range(_MAX_WAITS, len(waits), _MAX_WAITS):
            nxt = nc.sync.nop(nofuse=True)
            nxt.ins.sync_info = bass_rust.SyncInfo(
                on_wait=waits[i:i + _MAX_WAITS], on_update=[])
    nc.sync.drain()
    nc.all_engine_barrier()
    assert self.sems is not None
    popped = nc._tile_sem_poison_stack.pop()
    assert popped is self._sem_poison
    nc.clear_and_free_semaphores(list(self.sems.allocated().values()))
    nc.all_engine_barrier()


tile.TileContext._drain_and_barrier = _split_waits_drain_and_barrier

S, D, NCORES = 1536, 1024, 8
ST = S // 128            # 12 s-tiles per view
NB = 2 * ST              # 24 block rows of F
NCS = 2 * S // 512       # 6 column strips
KT = D // 128            # 8 contraction tiles
TEMP_INV = 20.0          # 1 / 0.05
FP8_SCALE = 8.0          # f entries ~N(0, 1/32); x8 keeps them in e4m3's
                         # normal range (|f|*8 <~ 2, well under 240)
F32 = mybir.dt.float32
BF16 = mybir.dt.bfloat16
FP8 = mybir.dt.float8e4
AF = mybir.ActivationFunctionType
ALU = mybir.AluOpType


def _build(num_devices: int = NCORES, debug_dump: bool = False) -> bass.Bass:
    nc = bass.Bass(num_devices=num_devices)
    h1 = nc.dram_tensor("h1", [S, D], F32, kind="ExternalInput")
    h2 = nc.dram_tensor("h2", [S, D], F32, kind="ExternalInput")
    # mask, pre-laid-out host-side as [128, ST] so token t = 128*col + row
    maskT = nc.dram_tensor("maskT", [128, ST], F32, kind="ExternalInput")
    out = nc.dram_tensor("loss", [1, 1], F32, kind="ExternalOutput")
    if debug_dump:
        ng_dump = nc.dram_tensor("ng_dump", [128, NB], F32,
                                 kind="ExternalOutput")
        cacc_dump = nc.dram_tensor("cacc_dump", [128, ST], F32,
                                   kind="ExternalOutput")

    exp_scale = TEMP_INV / (FP8_SCALE * FP8_SCALE)

    with tile.TileContext(nc) as tc, ExitStack() as ctx:
        const_pool = ctx.enter_context(tc.tile_pool(name="const", bufs=1))
        big = ctx.enter_context(tc.tile_pool(name="big", bufs=1))
        stat = ctx.enter_context(tc.tile_pool(name="stat", bufs=1))

        ident = const_pool.tile([128, 128], BF16)
        make_identity(nc, ident[:])
        ones_col = const_pool.tile([128, 1], F32)
        nc.gpsimd.memset(ones_col[:], 1.0)
        ones_sq = const_pool.tile([128, 128], F32)
        nc.gpsimd.memset(ones_sq[:], 1.0)
        ones_bf = const_pool.tile([128, 1], BF16)
        nc.gpsimd.memset(ones_bf[:], 1.0)
        msk = const_pool.tile([128, ST], F32)
        nc.sync.dma_start(msk[:], maskT[:])

        fT1 = big.tile([128, KT, S], FP8)        # f1^T * 8, fp8e4
        fT2 = big.tile([128, KT, S], FP8)        # f2^T * 8
        h1keep = big.tile([128, ST, D], F32)     # raw h1, kept for pos dot
        s12 = stat.tile([128, ST], F32)          # raw <h1_i, h2_i>
        poss = stat.tile([128, ST], F32)         # pos_sim (masked rows -> 0)
        sc1buf = stat.tile([128, ST], F32)       # masked rsqrt scales view1
        acc = stat.tile([128, NB, NCS], F32)     # per-strip row sums
        cacc = stat.tile([128, ST], F32)         # B column sums (view-2 Ng)
        msk24 = stat.tile([128, NB], F32)
        pose24 = stat.tile([128, NB], F32)       # exp(pos_sim/T), doubled
        poss20m = stat.tile([128, NB], F32)      # mask * pos_sim/T, doubled
        negK0 = stat.tile([128, 1], F32)
        recn = stat.tile([1, 1], F32)

        # acc[view2 rows, A-col-strips] is never written; zero everything
        nc.gpsimd.memset(acc[:], 0.0)

        # ---- phase 0: mask-only precomputes ----
        with tc.tile_pool(name="ep0", bufs=1) as ep0, \
             tc.tile_pool(name="ep0_ps", bufs=1, space="PSUM") as ep0p:
            msum = ep0.tile([128, 1], F32)
            nc.vector.tensor_reduce(msum[:], msk[:],
                                    axis=mybir.AxisListType.X, op=ALU.add)
            nps = ep0p.tile([128, 1], F32)
            nc.tensor.matmul(nps[:], ones_sq[:], msum[:], start=True,
                             stop=True)
            # -K0 = 2n - 2S
            nc.scalar.activation(negK0[:], nps[:], AF.Copy, scale=2.0,
                                 bias=float(-2 * S))
            n2c = ep0.tile([1, 1], F32)
            nc.scalar.activation(n2c[:], nps[0:1, :], AF.Copy,
                                 scale=float(2 * num_devices))
            nc.vector.reciprocal(recn[:], n2c[:])   # 1/(2 n ncores)
            nc.vector.tensor_copy(msk24[:, 0:ST], msk[:])
            nc.vector.tensor_copy(msk24[:, ST:NB], msk[:])

        def load_view(t, dst, src_dram, keep):
            if keep is not None:
                ht = keep
                nc.sync.dma_start(ht[:], src_dram[t * 128:(t + 1) * 128, :])
            else:
                nc.sync.dma_start(dst[:], src_dram[t * 128:(t + 1) * 128, :])
                ht = dst
            return ht

        def norms_of(scp, scr, ht, t, tag):
            sq = scr.tile([128, D], BF16, tag="sq", name=f"sq_{tag}_{t}")
            ss = scp.tile([128, 1], F32, tag="ss", name=f"ss_{tag}_{t}")
            nc.scalar.activation(sq[:], ht[:], AF.Square, accum_out=ss[:])
            nrm = scp.tile([128, 1], F32, tag="nrm", name=f"nrm_{tag}_{t}")
            nc.scalar.sqrt(nrm[:], ss[:])
            ri = scp.tile([128, 1], F32, tag="ri", name=f"ri_{tag}_{t}")
            nc.vector.reciprocal(ri[:], nrm[:])
            sc = scp.tile([128, 1], F32, tag="msc", name=f"sc_{tag}_{t}")
            nc.vector.tensor_mul(sc[:], ri[:], msk[:, t:t + 1])
            return sc

        def normalize_transpose(scr, tps, ht, sc, fT, t, tag):
            fn = scr.tile([128, D], BF16, tag="fn", name=f"fn_{tag}_{t}")
            nc.vector.tensor_scalar_mul(fn[:], ht[:], sc[:])
            c0 = t * 128
            for kg in range(2):
                pt = tps.tile([128, 512], BF16, tag="pt", name=f"pt_{tag}_{t}_{kg}")
                for j in range(4):
                    k = kg * 4 + j
                    nc.tensor.transpose(pt[:, j * 128:(j + 1) * 128],
                                        fn[:, k * 128:(k + 1) * 128],
                                        ident[:])
                nc.vector.tensor_scalar_mul(
                    fT[:, kg * 4:(kg + 1) * 4, c0:c0 + 128],
                    pt[:].rearrange("p (j c) -> p j c", j=4),
                    FP8_SCALE)

        def strip(mmp, esp, cs, r):
            """One [128,512] sim strip: matmuls, exp, row-sum into acc."""
            lhsT = fT1 if r < ST else fT2
            rT = r % ST
            rhsT = fT1 if cs < NCS // 2 else fT2
            csT = cs % (NCS // 2)
            ps = mmp.tile([128, 512], F32, tag="ps", name=f"ps_{cs}_{r}")
            for g in range(KT // 2):
                nc.tensor.matmul(
                    ps[:],
                    lhsT[:, 2 * g:2 * g + 2, rT * 128:(rT + 1) * 128],
                    rhsT[:, 2 * g:2 * g + 2, csT * 512:(csT + 1) * 512],
                    perf_mode=mybir.MatmulPerfMode.DoubleRow,
                    start=(g == 0), stop=(g == KT // 2 - 1))
            es = esp.tile([128, 512], BF16, tag="es", name=f"es_{cs}_{r}")
            bad = [bc for bc in (r % ST, r % ST + ST)
                   if cs * 4 <= bc < cs * 4 + 4]
            if bad:
                jb = bad[0] - cs * 4
                nc.scalar.activation(es[:], ps[:], AF.Exp, scale=exp_scale)
                blk = es[:, jb * 128:(jb + 1) * 128]
                nc.gpsimd.affine_select(
                    out=blk, in_=blk, compare_op=ALU.not_equal,
                    fill=0.0, base=0, pattern=[[-1, 128]],
                    channel_multiplier=1)
                nc.vector.tensor_reduce(acc[:, r, cs:cs + 1], es[:],
                                        axis=mybir.AxisListType.X,
                                        op=ALU.add)
            else:
                nc.scalar.activation(es[:], ps[:], AF.Exp, scale=exp_scale,
                                     accum_out=acc[:, r, cs:cs + 1])
            return es

        with tc.tile_pool(name="mm_ps", bufs=3, space="PSUM") as mmp, \
             tc.tile_pool(name="es", bufs=3) as esp, \
             tc.tile_pool(name="scr", bufs=2) as scr, \
             tc.tile_pool(name="sc", bufs=4) as scp:

            # ---- phase A: view-1 load/normalize/transpose ----
            with tc.tile_pool(name="tpA_ps", bufs=3, space="PSUM") as tps:
                for t in range(ST):
                    ht = load_view(t, None, h1, h1keep[:, t, :])
                    sc1 = norms_of(scp, scr, ht, t, "a")
                    nc.vector.tensor_copy(sc1buf[:, t:t + 1], sc1[:])
                    normalize_transpose(scr, tps, ht, sc1, fT1, t, "a")

            # ---- phase A': A-quadrant strips (only need view 1) ----
            for cs in range(NCS // 2):
                for r in range(ST):
                    strip(mmp, esp, cs, r)

            # ---- phase B: view-2 load/normalize/transpose + pos dot ----
            with tc.tile_pool(name="tpB_ps", bufs=3, space="PSUM") as tps, \
                 tc.tile_pool(name="ldB", bufs=3) as ldB:
                for t in range(ST):
                    tb = ldB.tile([128, D], F32, tag="h2", name=f"h2_{t}")
                    load_view(t, tb, h2, None)
                    sc2 = norms_of(scp, scr, tb, t, "b")
                    prod = scr.tile([128, D], F32, tag="prod",
                                    name=f"prod_{t}")
                    nc.vector.tensor_mul(prod[:], h1keep[:, t, :], tb[:])
                    nc.vector.tensor_reduce(s12[:, t:t + 1], prod[:],
                                            axis=mybir.AxisListType.X,
                                            op=ALU.add)
                    ptmp = scp.tile([128, 1], F32, tag="ptmp",
                                    name=f"ptmp_{t}")
                    nc.vector.tensor_mul(ptmp[:], s12[:, t:t + 1],
                                         sc1buf[:, t:t + 1])
                    nc.vector.tensor_mul(poss[:, t:t + 1], ptmp[:], sc2[:])
                    normalize_transpose(scr, tps, tb, sc2, fT2, t, "b")

            # pos-dependent epilogue precomputes (overlap with B/C strips)
            nc.scalar.activation(pose24[:, 0:ST], poss[:], AF.Exp,
                                 scale=TEMP_INV)
            nc.scalar.activation(pose24[:, ST:NB], poss[:], AF.Exp,
                                 scale=TEMP_INV)
            p20 = stat.tile([128, ST], F32)
            nc.scalar.mul(p20[:], poss[:], TEMP_INV)
            nc.vector.tensor_mul(poss20m[:, 0:ST], p20[:], msk[:])
            nc.vector.tensor_copy(poss20m[:, ST:NB], poss20m[:, 0:ST])

            # ---- phase B': B and C strips + B column sums ----
            with tc.tile_pool(name="cb_ps", bufs=1, space="PSUM") as cbp:
                for cs in range(NCS // 2, NCS):
                    pcb = []
                    for jb in range(4):
                        pcb_jb = cbp.tile([128, 1], F32, tag=f"cb{jb}",
                                          name=f"pcb_{cs}_{jb}")
                        pcb.append(pcb_jb)
                    for r in range(NB):
                        es = strip(mmp, esp, cs, r)
                        if r < ST:
                            for jb in range(4):
                                nc.tensor.matmul(
                                    pcb[jb][:],
                                    es[:, jb * 128:(jb + 1) * 128],
                                    ones_bf[:],
                                    start=(r == 0), stop=(r == ST - 1),
                                    skip_group_check=True)
                    c0 = (cs - NCS // 2) * 4
                    for jb in range(4):
                        nc.vector.tensor_copy(cacc[:, c0 + jb:c0 + jb + 1],
                                              pcb[jb][:])

        # ---- phase C: final reduction chain ----
        with tc.tile_pool(name="ep", bufs=1) as ep, \
             tc.tile_pool(name="ep_ps", bufs=1, space="PSUM") as epp:
            ng = ep.tile([128, NB], F32)
            nc.vector.tensor_reduce(ng[:], acc[:], axis=mybir.AxisListType.X,
                                    op=ALU.add)
            nc.vector.tensor_add(ng[:, ST:NB], ng[:, ST:NB], cacc[:])
            if debug_dump:
                nc.sync.dma_start(ng_dump[:], ng[:])
                nc.sync.dma_start(cacc_dump[:], cacc[:])
            denom = ep.tile([128, NB], F32)
            nc.vector.tensor_scalar_add(denom[:], ng[:], negK0[:])
            nc.vector.tensor_add(denom[:], denom[:], pose24[:])
            lg = ep.tile([128, NB], F32)
            nc.scalar.activation(lg[:], denom[:], AF.Ln)
            ptok = ep.tile([128, NB], F32)
            nc.vector.tensor_mul(ptok[:], lg[:], msk24[:])
            nc.vector.tensor_sub(ptok[:], ptok[:], poss20m[:])
            tsum = ep.tile([128, 1], F32)
            nc.vector.tensor_reduce(tsum[:], ptok[:],
                                    axis=mybir.AxisListType.X, op=ALU.add)
            lps = epp.tile([1, 1], F32)
            nc.tensor.matmul(lps[:], ones_col[:], tsum[:], start=True,
                             stop=True)
            lsb = ep.tile([1, 1], F32)
            nc.vector.tensor_mul(lsb[:], lps[:], recn[:])

            with tc.tile_pool(name="dram", bufs=1, space="DRAM") as dram:
                if num_devices > 1:
                    lin = dram.tile([1, 1], F32)
                    lout = dram.tile([1, 1], F32)
                    nc.sync.dma_start(lin[:], lsb[:])
                    nc.gpsimd.collective_compute(
                        "AllReduce", ALU.add,
                        replica_groups=[list(range(num_devices))],
                        ins=[lin.opt()], outs=[lout.opt()])
                    nc.sync.dma_start(out[:], lout[:])
                else:
                    nc.sync.dma_start(out[:], lsb[:])

    return nc


_NC = None


def _mask_layout(mask_row: np.ndarray) -> np.ndarray:
    # token t = 128 * col + row  ->  [128, ST]
    return np.ascontiguousarray(
        mask_row.astype(np.float32).reshape(ST, 128).T)


def kernel(last_hidden_states_1, last_hidden_states_2, token_mask_batch):
    global _NC
    h1 = np.ascontiguousarray(np.asarray(last_hidden_states_1,
                                         dtype=np.float32))
    h2 = np.ascontiguousarray(np.asarray(last_hidden_states_2,
                                         dtype=np.float32))
    mask = np.asarray(token_mask_batch)
    assert h1.shape == (NCORES, S, D), h1.shape

    if _NC is None:
        _NC = _build(NCORES)

    in_maps = [
        {"h1": h1[b], "h2": h2[b], "maskT": _mask_layout(mask[b])}
        for b in range(NCORES)
    ]
    res = run_bass_kernel_spmd(_NC, in_maps, list(range(NCORES)))
    loss = np.asarray(res.results[0]["loss"], dtype=np.float32).reshape(())
    return loss



# revision 6
# speedup vs baseline: 1.6983x; 1.6983x over previous
"""ContraCLM token-level contrastive loss on 8 Trainium2 NeuronCores.

Data-parallel over the batch: core b handles sample b (B=8). Host-side,
each sample's unmasked tokens are compacted to the front (a pure gather /
layout transform; the kernel still sees real data rows for pads so norms
never hit 0/0) and padded to P=1024 (n ~ Binomial(1536, .5) ~ 768; the
build is generic in P with a P=1536 fallback if some n > 1024).

Per core, with P=1024, D=1024, T=0.05:

  f_v = l2norm(h_v) with pad rows zeroed (mask folded into the reciprocal
  norm scale); G_v = (8*f_v)^T stored [D, P] in fp8e4 (x8 keeps entries
  in e4m3's normal range).

  The 2P x 2P similarity matrix [[A B];[B^T C]] (A = f1 f1^T etc.) is
  symmetric, so only A/C upper-triangle strips and all of B are computed
  as [128, 512] PSUM strips (fp8 DoubleRow, K=1024). exp(sim/T) row sums
  come from the ScalarE activation free-dim accumulator; strips containing
  the diagonal get a strict-upper affine_select then a DVE row-sum. The
  mirrored (lower-triangle) contributions are recovered from column sums:
  a ones-vector stationary matmul streams each es strip into a per-column
  [1, 512] PSUM accumulator, which is transposed to token-major layout at
  the end via K=1 outer-product matmuls.

  B's diagonal is exp(pos_sim/T): it is left inside the row/col sums
  (denominator = Ng + pos exactly), and 20*pos_sim for the numerator is
  extracted exactly from the f32 PSUM sim diagonal with a fused
  tensor_tensor_reduce against an identity tile.

  Pad columns contribute exp(0)=1 to every row sum: subtract
  K0 = 2P - 2n. per_tok = ln(denom) - 20*pos_sim, masked mean over 2n
  tokens; per-sample means are averaged on the host (no collective).
"""

import sys

for _p in ("/opt/trn_rl_repo", "/opt/pypackages"):
    if _p not in sys.path:
        sys.path.append(_p)

from contextlib import ExitStack

import numpy as np

import bass_rust

import concourse.bass as bass
import concourse.tile as tile
from concourse import mybir
from concourse.bass_utils import run_bass_kernel_spmd
from concourse.masks import make_identity
from concourse.vector_clock import ScopedClock

# The walrus build in this container encodes at most 2 sync waits per
# instruction (bass_rust's inst_waits_full agrees), but Tile's semaphore
# assignment can attach more. Hoist excess waits onto unfusable same-engine
# NoOps immediately before the instruction — the engine executes its queue
# in order, so semantics are preserved.
_MAX_WAITS = 1


def _split_excess_waits(nc, ordered):
    for bb_name, insts in ordered.items():
        out = []
        changed = False
        for inst in insts:
            si = getattr(inst, "sync_info", None)
            waits = list(si.on_wait) if si is not None else []
            if len(waits) > _MAX_WAITS:
                changed = True
                extra, keep = waits[:-_MAX_WAITS], waits[-_MAX_WAITS:]
                for i in range(0, len(extra), _MAX_WAITS):
                    out.append(mybir.InstNoOp(
                        name=nc.get_next_instruction_name(),
                        sync_info=mybir.SyncInfo(
                            on_wait=extra[i:i + _MAX_WAITS], on_update=[]),
                        bass_nofuse=True,
                        engine=inst.engine,
                    ))
                si.on_wait = keep
            out.append(inst)
        if changed:
            insts[:] = out


_orig_lower_ordered_insts = tile.TileContext._lower_ordered_insts


def _patched_lower_ordered_insts(self, ordered):
    _split_excess_waits(self.nc, ordered)
    return _orig_lower_ordered_insts(self, ordered)


tile.TileContext._lower_ordered_insts = _patched_lower_ordered_insts


def _split_waits_drain_and_barrier(self, tick_clock, wait_clock):
    nc = self.nc
    probe = nc.sync.nop(nofuse=True)
    wait_clock.add_sem_waits(
        probe.ins, ScopedClock({None: tick_clock.global_clock}))
    si = probe.ins.sync_info
    waits = list(si.on_wait) if si is not None else []
    if len(waits) > _MAX_WAITS:
        si.on_wait = waits[:_MAX_WAITS]
        for i in range(_MAX_WAITS, len(waits), _MAX_WAITS):
            nxt = nc.sync.nop(nofuse=True)
            nxt.ins.sync_info = bass_rust.SyncInfo(
                on_wait=waits[i:i + _MAX_WAITS], on_update=[])
    nc.sync.drain()
    nc.all_engine_barrier()
    assert self.sems is not None
    popped = nc._tile_sem_poison_stack.pop()
    assert popped is self._sem_poison
    nc.clear_and_free_semaphores(list(self.sems.allocated().values()))
    nc.all_engine_barrier()


tile.TileContext._drain_and_barrier = _split_waits_drain_and_barrier

S, D, NCORES = 1536, 1024, 8
P_MAIN = 1024            # compacted+padded tokens per view
KT = D // 128            # 8 contraction tiles
TEMP_INV = 20.0          # 1 / 0.05
FP8_SCALE = 8.0          # f entries ~N(0, 1/32); x8 keeps them in e4m3's
                         # normal range (|f|*8 <~ 2, well under 240)
F32 = mybir.dt.float32
BF16 = mybir.dt.bfloat16
FP8 = mybir.dt.float8e4
AF = mybir.ActivationFunctionType
ALU = mybir.AluOpType
DR = mybir.MatmulPerfMode.DoubleRow


def _build(p: int, num_devices: int = NCORES) -> bass.Bass:
    PT = p // 128        # token tiles per view
    NB2 = 2 * PT         # block rows of the 2P x 2P matrix
    half = p // 512      # column strips per view
    exp_scale = TEMP_INV / (FP8_SCALE * FP8_SCALE)
    pos_scale = TEMP_INV / (FP8_SCALE * FP8_SCALE)

    nc = bass.Bass(num_devices=num_devices)
    h1 = nc.dram_tensor("h1", [p, D], F32, kind="ExternalInput")
    h2 = nc.dram_tensor("h2", [p, D], F32, kind="ExternalInput")
    # mask, pre-laid-out host-side as [128, PT] so token t = 128*col + row
    maskT = nc.dram_tensor("maskT", [128, PT], F32, kind="ExternalInput")
    out = nc.dram_tensor("loss", [1, 1], F32, kind="ExternalOutput")

    with tile.TileContext(nc) as tc, ExitStack() as ctx:
        const_pool = ctx.enter_context(tc.tile_pool(name="const", bufs=1))
        big = ctx.enter_context(tc.tile_pool(name="big", bufs=1))
        stat = ctx.enter_context(tc.tile_pool(name="stat", bufs=1))

        identF8 = const_pool.tile([128, 128], FP8)
        make_identity(nc, identF8[:])
        identB = const_pool.tile([128, 128], BF16)
        make_identity(nc, identB[:])
        ones_col = const_pool.tile([128, 1], F32)
        nc.gpsimd.memset(ones_col[:], 1.0)
        ones_sq = const_pool.tile([128, 128], F32)
        nc.gpsimd.memset(ones_sq[:], 1.0)
        ones_bf = const_pool.tile([128, 1], BF16)
        nc.gpsimd.memset(ones_bf[:], 1.0)
        one_f32 = const_pool.tile([1, 1], F32)
        nc.gpsimd.memset(one_f32[:], 1.0)
        msk = const_pool.tile([128, PT], F32)
        nc.sync.dma_start(msk[:], maskT[:])
        msk8 = const_pool.tile([128, PT], F32)
        nc.scalar.mul(msk8[:], msk[:], FP8_SCALE)

        fT1 = big.tile([128, KT, p], FP8)        # (8*f1)^T, fp8e4
        fT2 = big.tile([128, KT, p], FP8)        # (8*f2)^T
        acc = stat.tile([128, NB2, 2 * half], F32)   # per-strip row sums
        poss20 = stat.tile([128, PT], F32)       # 20 * pos_sim
        csum_sb = stat.tile([1, 2 * p], F32)     # mirror column sums
        msk24 = stat.tile([128, NB2], F32)
        negK0 = stat.tile([128, 1], F32)
        recn = stat.tile([1, 1], F32)

        # zero row-sum slots never written (below-diagonal A/C strips)
        nc.gpsimd.memset(acc[:], 0.0)

        # ---- HAM warmup: keep the PE busy from t~0 so it upclocks ----
        with tc.tile_pool(name="warm", bufs=1, space="PSUM") as wp:
            wps = wp.tile([128, 128], F32)
            for _ in range(24):
                nc.tensor.matmul(wps[:], identB[:], identB[:],
                                 start=True, stop=True)

        # ---- phase 0: mask-only precomputes ----
        with tc.tile_pool(name="ep0", bufs=1) as ep0, \
             tc.tile_pool(name="ep0_ps", bufs=1, space="PSUM") as ep0p:
            msum = ep0.tile([128, 1], F32)
            nc.vector.tensor_reduce(msum[:], msk[:],
                                    axis=mybir.AxisListType.X, op=ALU.add)
            nps = ep0p.tile([128, 1], F32)
            nc.tensor.matmul(nps[:], ones_sq[:], msum[:], start=True,
                             stop=True)
            # -K0 = 2n - 2P, broadcast to all partitions
            nc.scalar.activation(negK0[:], nps[:], AF.Copy, scale=2.0,
                                 bias=float(-2 * p))
            n2c = ep0.tile([1, 1], F32)
            nc.scalar.activation(n2c[:], nps[0:1, :], AF.Copy, scale=2.0)
            nc.vector.reciprocal(recn[:], n2c[:])   # 1/(2n)
            nc.vector.tensor_copy(msk24[:, 0:PT], msk[:])
            nc.vector.tensor_copy(msk24[:, PT:NB2], msk[:])

        with tc.tile_pool(name="mm_ps", bufs=3, space="PSUM") as mmp, \
             tc.tile_pool(name="cng_ps", bufs=1, space="PSUM") as cngp, \
             tc.tile_pool(name="es", bufs=4) as esp, \
             tc.tile_pool(name="scr", bufs=2) as scr, \
             tc.tile_pool(name="sc", bufs=4) as scp, \
             tc.tile_pool(name="tt", bufs=2) as ttp:

            cng = cngp.tile([128, NB2], F32)     # mirror sums, token-major
            pending = []                         # deferred colsum matmuls

            def flush_pending():
                while pending:
                    pending.pop(0)()

            def emit_tile(v, t, src_dram, fT, tps):
                ht = scr.tile([128, D], F32, tag="ht", name=f"ht{v}_{t}")
                nc.sync.dma_start(ht[:], src_dram[t * 128:(t + 1) * 128, :])
                sq = scr.tile([128, D], BF16, tag="sq", name=f"sq{v}_{t}")
                ss = scp.tile([128, 1], F32, tag="ss", name=f"ss{v}_{t}")
                nc.scalar.activation(sq[:], ht[:], AF.Square, accum_out=ss[:])
                nrm = scp.tile([128, 1], F32, tag="nrm", name=f"nrm{v}_{t}")
                nc.scalar.sqrt(nrm[:], ss[:])
                ri = scp.tile([128, 1], F32, tag="ri", name=f"ri{v}_{t}")
                nc.vector.reciprocal(ri[:], nrm[:])
                sc8 = scp.tile([128, 1], F32, tag="sc8", name=f"sc8{v}_{t}")
                nc.vector.tensor_mul(sc8[:], ri[:], msk8[:, t:t + 1])
                fn8 = scr.tile([128, D], FP8, tag="fn", name=f"fn{v}_{t}")
                nc.vector.tensor_scalar_mul(fn8[:], ht[:], sc8[:])
                # fp8 transpose-mode matmuls require the PSUM output AP to
                # have element step 2 (fp8 interleaved in PSUM)
                pt = tps.tile([128, KT, 128, 2], FP8, tag="pt",
                              name=f"pt{v}_{t}")
                for k in range(KT):
                    nc.tensor.transpose(pt[:, k, :, 0:1],
                                        fn8[:, k * 128:(k + 1) * 128],
                                        identF8[:])
                nc.scalar.activation(
                    fT[:, :, t * 128:(t + 1) * 128],
                    pt[:, :, :, 0],
                    AF.Copy)

            def emit_strip(r, csT, quad, cs_ps, first, last):
                """One [128,512] sim strip at block-row r (global), local
                column strip csT of quadrant quad."""
                lhsT = fT1 if r < PT else fT2
                rT = r % PT
                rhsT = fT1 if quad == "A" else fT2
                ps = mmp.tile([128, 512], F32, tag="ps",
                              name=f"ps{quad}_{csT}_{r}")
                for g in range(KT // 2):
                    nc.tensor.matmul(
                        ps[:],
                        lhsT[:, 2 * g:2 * g + 2, rT * 128:(rT + 1) * 128],
                        rhsT[:, 2 * g:2 * g + 2, csT * 512:(csT + 1) * 512],
                        perf_mode=DR,
                        start=(g == 0), stop=(g == KT // 2 - 1))
                # previous strip's colsum lands here: by now its exp result
                # is ready, so the PE doesn't stall on it
                flush_pending()
                es = esp.tile([128, 512], BF16, tag="es",
                              name=f"es{quad}_{csT}_{r}")
                cs_g = csT if quad == "A" else half + csT
                if quad == "B":
                    nc.scalar.activation(es[:], ps[:], AF.Exp,
                                         scale=exp_scale,
                                         accum_out=acc[:, r, cs_g:cs_g + 1])
                    if csT * 4 <= rT <= csT * 4 + 3:
                        # pos_sim lives on this strip's diagonal block:
                        # extract it exactly from the f32 PSUM sims
                        jb = rT - csT * 4
                        sct = ttp.tile([128, 128], F32, tag="sct",
                                       name=f"sct_{r}")
                        nc.vector.scalar_tensor_tensor(
                            out=sct[:],
                            in0=ps[:, jb * 128:(jb + 1) * 128],
                            scalar=pos_scale,
                            in1=identB[:],
                            op0=ALU.mult,
                            op1=ALU.mult,
                            accum_out=poss20[:, rT:rT + 1])
                else:
                    # A/C quadrants: keep only the strict upper triangle
                    K = csT * 512 - rT * 128
                    if K >= 128:
                        nc.scalar.activation(
                            es[:], ps[:], AF.Exp, scale=exp_scale,
                            accum_out=acc[:, r, cs_g:cs_g + 1])
                    else:
                        nc.scalar.activation(es[:], ps[:], AF.Exp,
                                             scale=exp_scale)
                        # keep col > row: base + (-1)*p + 1*c >= 0
                        nc.gpsimd.affine_select(
                            out=es[:], in_=es[:], compare_op=ALU.is_ge,
                            fill=0.0, base=K - 1, pattern=[[1, 512]],
                            channel_multiplier=-1)
                        nc.vector.tensor_reduce(acc[:, r, cs_g:cs_g + 1],
                                                es[:],
                                                axis=mybir.AxisListType.X,
                                                op=ALU.add)

                def colsum(es=es, first=first, last=last, vec=cs_ps):
                    nc.tensor.matmul(vec[0:1, :], ones_bf[:], es[:],
                                     start=first, stop=last,
                                     skip_group_check=True)
                pending.append(colsum)

            def emit_mirror(base_chunk, nchunks):
                # transpose csum_sb chunks to token-major via K=1 matmuls
                for c in range(base_chunk, base_chunk + nchunks):
                    nc.tensor.matmul(cng[:, c:c + 1],
                                     csum_sb[0:1, c * 128:(c + 1) * 128],
                                     one_f32[:], start=True, stop=True)

            # ---- phase A: view 1 tiles + A-quadrant strips ----
            with tc.tile_pool(name="tpA_ps", bufs=2, space="PSUM") as tps, \
                 tc.tile_pool(name="psA", bufs=1, space="PSUM") as psAp:
                psA = [psAp.tile([1, 512], F32, name=f"psA{c}")
                       for c in range(half)]
                for t in range(PT):
                    emit_tile(1, t, h1, fT1, tps)
                    flush_pending()
                    if t % 4 == 3:
                        csT = t // 4
                        rows = list(range(min(4 * csT + 4, PT)))
                        for r in rows:
                            emit_strip(r, csT, "A", psA[csT],
                                       first=(r == rows[0]),
                                       last=(r == rows[-1]))
                flush_pending()
                for c in range(half):
                    nc.vector.tensor_copy(
                        csum_sb[0:1, c * 512:(c + 1) * 512], psA[c][:])
            emit_mirror(0, PT)

            # ---- phase B: view 2 tiles + B/C strips ----
            with tc.tile_pool(name="tpB_ps", bufs=2, space="PSUM") as tps, \
                 tc.tile_pool(name="psB", bufs=1, space="PSUM") as psBp:
                psB = [psBp.tile([1, 512], F32, name=f"psB{c}")
                       for c in range(half)]
                for t in range(PT):
                    emit_tile(2, t, h2, fT2, tps)
                    flush_pending()
                    if t % 4 == 3:
                        csT = t // 4
                        # B strips: all view-1 rows
                        for r in range(PT):
                            emit_strip(r, csT, "B", psB[csT],
                                       first=(r == 0), last=False)
                        # C strips: upper triangle rows
                        crows = list(range(min(4 * csT + 4, PT)))
                        for rc in crows:
                            emit_strip(PT + rc, csT, "C", psB[csT],
                                       first=False, last=(rc == crows[-1]))
                flush_pending()
                for c in range(half):
                    nc.vector.tensor_copy(
                        csum_sb[0:1, p + c * 512:p + (c + 1) * 512],
                        psB[c][:])
            emit_mirror(PT, PT)

            # ---- epilogue: final reduction chain ----
            with tc.tile_pool(name="ep", bufs=1) as ep, \
                 tc.tile_pool(name="ep_ps", bufs=1, space="PSUM") as epp:
                cngs = ep.tile([128, NB2], F32)
                nc.vector.tensor_copy(cngs[:], cng[:])
                ng = ep.tile([128, NB2], F32)
                nc.vector.tensor_reduce(ng[:], acc[:],
                                        axis=mybir.AxisListType.X,
                                        op=ALU.add)
                den = ep.tile([128, NB2], F32)
                nc.vector.tensor_add(den[:], ng[:], cngs[:])
                nc.vector.tensor_scalar_add(den[:], den[:], negK0[:])
                lg = ep.tile([128, NB2], F32)
                nc.scalar.activation(lg[:], den[:], AF.Ln)
                pm = ep.tile([128, NB2], F32)
                nc.vector.tensor_copy(pm[:, 0:PT], poss20[:])
                nc.vector.tensor_copy(pm[:, PT:NB2], poss20[:])
                d1 = ep.tile([128, NB2], F32)
                nc.vector.tensor_sub(d1[:], lg[:], pm[:])
                ptok = ep.tile([128, NB2], F32)
                tsum = ep.tile([128, 1], F32)
                nc.vector.scalar_tensor_tensor(
                    out=ptok[:], in0=d1[:], scalar=1.0, in1=msk24[:],
                    op0=ALU.mult, op1=ALU.mult, accum_out=tsum[:])
                lps = epp.tile([1, 1], F32)
                nc.tensor.matmul(lps[:], ones_col[:], tsum[:], start=True,
                                 stop=True)
                lsb = ep.tile([1, 1], F32)
                nc.vector.tensor_mul(lsb[:], lps[:], recn[:])
                nc.sync.dma_start(out[:], lsb[:])

    return nc


_NC = {}


def _get_nc(p: int) -> bass.Bass:
    if p not in _NC:
        _NC[p] = _build(p)
    return _NC[p]


def _mask_layout(mask_col: np.ndarray, p: int) -> np.ndarray:
    # token t = 128 * col + row  ->  [128, PT]
    return np.ascontiguousarray(
        mask_col.astype(np.float32).reshape(p // 128, 128).T)


def _in_maps(h1, h2, mask, p):
    maps = []
    for b in range(NCORES):
        idx = np.argsort(~mask[b], kind="stable")[:p]
        maps.append({
            "h1": np.ascontiguousarray(h1[b][idx]),
            "h2": np.ascontiguousarray(h2[b][idx]),
            "maskT": _mask_layout(mask[b][idx], p),
        })
    return maps


def kernel(last_hidden_states_1, last_hidden_states_2, token_mask_batch):
    h1 = np.ascontiguousarray(np.asarray(last_hidden_states_1,
                                         dtype=np.float32))
    h2 = np.ascontiguousarray(np.asarray(last_hidden_states_2,
                                         dtype=np.float32))
    mask = np.asarray(token_mask_batch).astype(bool)
    assert h1.shape == (NCORES, S, D), h1.shape

    p = P_MAIN if int(mask.sum(axis=1).max()) <= P_MAIN else S
    nc = _get_nc(p)
    res = run_bass_kernel_spmd(nc, _in_maps(h1, h2, mask, p),
                               list(range(NCORES)))
    vals = [np.asarray(res.results[b]["loss"], dtype=np.float32).reshape(())
            for b in range(NCORES)]
    return np.float32(np.mean(vals))


# revision 15
# speedup vs baseline: 1.8915x; 1.1138x over previous
"""ContraCLM token-level contrastive loss on 8 Trainium2 NeuronCores.

Data-parallel over the batch: core b handles sample b (B=8). Host-side,
each sample's unmasked tokens are compacted to the front (a pure gather /
layout transform; the kernel still sees real data rows for pads so norms
never hit 0/0) and padded to P=1024 (n ~ Binomial(1536, .5) ~ 768; the
build is generic in P with a P=1536 fallback if some n > 1024).

Per core, with P=1024, D=1024, T=0.05:

  f_v = l2norm(h_v) with pad rows zeroed (mask folded into the reciprocal
  norm scale); G_v = (8*f_v)^T stored [D, P] in fp8e4 (x8 keeps entries
  in e4m3's normal range).

  The 2P x 2P similarity matrix [[A B];[B^T C]] (A = f1 f1^T etc.) is
  symmetric, so only A/C upper-triangle strips and all of B are computed
  as [128, 512] PSUM strips (fp8 DoubleRow, K=1024). exp(sim/T) row sums
  come from the ScalarE activation free-dim accumulator; strips containing
  the diagonal get a strict-upper affine_select then a DVE row-sum. The
  mirrored (lower-triangle) contributions are recovered from column sums:
  a ones-vector stationary matmul streams each es strip into a per-column
  [1, 512] PSUM accumulator, which is transposed to token-major layout at
  the end via K=1 outer-product matmuls.

  B's diagonal is exp(pos_sim/T): it is left inside the row/col sums
  (denominator = Ng + pos exactly), and 20*pos_sim for the numerator is
  extracted exactly from the f32 PSUM sim diagonal with a fused
  tensor_tensor_reduce against an identity tile.

  Pad columns contribute exp(0)=1 to every row sum: subtract
  K0 = 2P - 2n. per_tok = ln(denom) - 20*pos_sim, masked mean over 2n
  tokens; per-sample means are averaged on the host (no collective).
"""

import sys

for _p in ("/opt/trn_rl_repo", "/opt/pypackages"):
    if _p not in sys.path:
        sys.path.append(_p)

from contextlib import ExitStack

import numpy as np

import bass_rust

import concourse.bass as bass
import concourse.tile as tile
from concourse import mybir
from concourse.bass_utils import run_bass_kernel_spmd
from concourse.masks import make_identity
from concourse.vector_clock import ScopedClock

# The walrus build in this container encodes at most 2 sync waits per
# instruction (bass_rust's inst_waits_full agrees), but Tile's semaphore
# assignment can attach more. Hoist excess waits onto unfusable same-engine
# NoOps immediately before the instruction — the engine executes its queue
# in order, so semantics are preserved.
_MAX_WAITS = 1


def _split_excess_waits(nc, ordered):
    for bb_name, insts in ordered.items():
        out = []
        changed = False
        for inst in insts:
            si = getattr(inst, "sync_info", None)
            waits = list(si.on_wait) if si is not None else []
            if len(waits) > _MAX_WAITS:
                changed = True
                extra, keep = waits[:-_MAX_WAITS], waits[-_MAX_WAITS:]
                for i in range(0, len(extra), _MAX_WAITS):
                    out.append(mybir.InstNoOp(
                        name=nc.get_next_instruction_name(),
                        sync_info=mybir.SyncInfo(
                            on_wait=extra[i:i + _MAX_WAITS], on_update=[]),
                        bass_nofuse=True,
                        engine=inst.engine,
                    ))
                si.on_wait = keep
            out.append(inst)
        if changed:
            insts[:] = out


_orig_lower_ordered_insts = tile.TileContext._lower_ordered_insts


def _patched_lower_ordered_insts(self, ordered):
    _split_excess_waits(self.nc, ordered)
    return _orig_lower_ordered_insts(self, ordered)


tile.TileContext._lower_ordered_insts = _patched_lower_ordered_insts


def _split_waits_drain_and_barrier(self, tick_clock, wait_clock):
    nc = self.nc
    probe = nc.sync.nop(nofuse=True)
    wait_clock.add_sem_waits(
        probe.ins, ScopedClock({None: tick_clock.global_clock}))
    si = probe.ins.sync_info
    waits = list(si.on_wait) if si is not None else []
    if len(waits) > _MAX_WAITS:
        si.on_wait = waits[:_MAX_WAITS]
        for i in range(_MAX_WAITS, len(waits), _MAX_WAITS):
            nxt = nc.sync.nop(nofuse=True)
            nxt.ins.sync_info = bass_rust.SyncInfo(
                on_wait=waits[i:i + _MAX_WAITS], on_update=[])
    nc.sync.drain()
    nc.all_engine_barrier()
    assert self.sems is not None
    popped = nc._tile_sem_poison_stack.pop()
    assert popped is self._sem_poison
    nc.clear_and_free_semaphores(list(self.sems.allocated().values()))
    nc.all_engine_barrier()


tile.TileContext._drain_and_barrier = _split_waits_drain_and_barrier

S, D, NCORES = 1536, 1024, 8
P_MAIN = 1024            # compacted+padded tokens per view
KT = D // 128            # 8 contraction tiles
TEMP_INV = 20.0          # 1 / 0.05
FP8_SCALE = 8.0          # f entries ~N(0, 1/32); x8 keeps them in e4m3's
                         # normal range (|f|*8 <~ 2, well under 240)
F32 = mybir.dt.float32
BF16 = mybir.dt.bfloat16
FP8 = mybir.dt.float8e4
AF = mybir.ActivationFunctionType
ALU = mybir.AluOpType
DR = mybir.MatmulPerfMode.DoubleRow


def _build(p: int, num_devices: int = NCORES) -> bass.Bass:
    PT = p // 128        # token tiles per view
    NB2 = 2 * PT         # block rows of the 2P x 2P matrix
    half = p // 512      # column strips per view
    exp_scale = TEMP_INV / (FP8_SCALE * FP8_SCALE)
    pos_scale = TEMP_INV / (FP8_SCALE * FP8_SCALE)

    nc = bass.Bass(num_devices=num_devices)
    h1 = nc.dram_tensor("h1", [p, D], F32, kind="ExternalInput")
    h2 = nc.dram_tensor("h2", [p, D], F32, kind="ExternalInput")
    # mask, pre-laid-out host-side as [128, PT] so token t = 128*col + row
    maskT = nc.dram_tensor("maskT", [128, PT], F32, kind="ExternalInput")
    out = nc.dram_tensor("loss", [1, 1], F32, kind="ExternalOutput")

    with tile.TileContext(nc) as tc, ExitStack() as ctx:
        const_pool = ctx.enter_context(tc.tile_pool(name="const", bufs=1))
        big = ctx.enter_context(tc.tile_pool(name="big", bufs=1))
        stat = ctx.enter_context(tc.tile_pool(name="stat", bufs=1))

        identB = const_pool.tile([128, 128], BF16)
        make_identity(nc, identB[:])
        ones_col = const_pool.tile([128, 1], F32)
        nc.gpsimd.memset(ones_col[:], 1.0)
        ones_sq = const_pool.tile([128, 128], F32)
        nc.gpsimd.memset(ones_sq[:], 1.0)
        ones_bf = const_pool.tile([128, 1], BF16)
        nc.gpsimd.memset(ones_bf[:], 1.0)
        one_f32 = const_pool.tile([1, 1], F32)
        nc.gpsimd.memset(one_f32[:], 1.0)
        msk = const_pool.tile([128, PT], F32)
        nc.sync.dma_start(msk[:], maskT[:])

        fT1 = big.tile([128, KT, p], FP8)        # (8*f1)^T, fp8e4
        fT2 = big.tile([128, KT, p], FP8)        # (8*f2)^T
        acc = stat.tile([128, NB2, 2 * half], F32)   # per-strip row sums
        poss20 = stat.tile([128, PT], F32)       # 20 * pos_sim
        csum_sb = stat.tile([1, 2 * p], F32)     # mirror column sums
        msk24 = stat.tile([128, NB2], F32)
        negK0 = stat.tile([128, 1], F32)
        recn = stat.tile([1, 1], F32)

        # zero row-sum slots never written (below-diagonal A/C strips)
        nc.gpsimd.memset(acc[:], 0.0)

        # ---- HAM warmup: keep the PE busy from t~0 so it upclocks ----
        with tc.tile_pool(name="warm", bufs=1, space="PSUM") as wp:
            wps = wp.tile([128, 128], F32)
            for _ in range(24):
                nc.tensor.matmul(wps[:], identB[:], identB[:],
                                 start=True, stop=True)

        # ---- phase 0: mask-only precomputes ----
        with tc.tile_pool(name="ep0", bufs=1) as ep0, \
             tc.tile_pool(name="ep0_ps", bufs=1, space="PSUM") as ep0p:
            msum = ep0.tile([128, 1], F32)
            nc.vector.tensor_reduce(msum[:], msk[:],
                                    axis=mybir.AxisListType.X, op=ALU.add)
            nps = ep0p.tile([128, 1], F32)
            nc.tensor.matmul(nps[:], ones_sq[:], msum[:], start=True,
                             stop=True)
            # -K0 = 2n - 2P, broadcast to all partitions
            nc.scalar.activation(negK0[:], nps[:], AF.Copy, scale=2.0,
                                 bias=float(-2 * p))
            n2c = ep0.tile([1, 1], F32)
            nc.scalar.activation(n2c[:], nps[0:1, :], AF.Copy, scale=2.0)
            nc.vector.reciprocal(recn[:], n2c[:])   # 1/(2n)
            nc.vector.tensor_copy(msk24[:, 0:PT], msk[:])
            nc.vector.tensor_copy(msk24[:, PT:NB2], msk[:])

        with tc.tile_pool(name="mm_ps", bufs=3, space="PSUM") as mmp, \
             tc.tile_pool(name="cng_ps", bufs=1, space="PSUM") as cngp, \
             tc.tile_pool(name="es", bufs=4) as esp, \
             tc.tile_pool(name="ht", bufs=3) as htp, \
             tc.tile_pool(name="scr", bufs=2) as scr, \
             tc.tile_pool(name="sc", bufs=4) as scp, \
             tc.tile_pool(name="tt", bufs=2) as ttp:

            cng = cngp.tile([128, NB2], F32)     # mirror sums, token-major
            pending = []                         # deferred colsum matmuls

            def flush_pending():
                while pending:
                    pending.pop(0)()

            def emit_tile(v, t, src_dram, fT, tps):
                ht = htp.tile([128, D], F32, tag="ht", name=f"ht{v}_{t}")
                nc.sync.dma_start(ht[:], src_dram[t * 128:(t + 1) * 128, :])
                sq = scr.tile([128, D], BF16, tag="sq", name=f"sq{v}_{t}")
                ss = scp.tile([128, 1], F32, tag="ss", name=f"ss{v}_{t}")
                nc.scalar.activation(sq[:], ht[:], AF.Square, accum_out=ss[:])
                nrm = scp.tile([128, 1], F32, tag="nrm", name=f"nrm{v}_{t}")
                nc.scalar.sqrt(nrm[:], ss[:])
                ri = scp.tile([128, 1], F32, tag="ri", name=f"ri{v}_{t}")
                nc.vector.reciprocal(ri[:], nrm[:])
                scm = scp.tile([128, 1], F32, tag="scm", name=f"scm{v}_{t}")
                nc.vector.tensor_mul(scm[:], ri[:], msk[:, t:t + 1])
                fnb = scr.tile([128, D], BF16, tag="fn", name=f"fn{v}_{t}")
                nc.vector.tensor_scalar_mul(fnb[:], ht[:], scm[:])
                pt = tps.tile([128, D], BF16, tag="pt", name=f"pt{v}_{t}")
                for k in range(KT):
                    nc.tensor.transpose(pt[:, k * 128:(k + 1) * 128],
                                        fnb[:, k * 128:(k + 1) * 128],
                                        identB[:])
                # quantize to fp8 (x8) while moving PSUM->SBUF; alternate
                # engines so neither Scalar nor Vector becomes the choke
                dst = fT[:, :, t * 128:(t + 1) * 128]
                src = pt[:].rearrange("q (k c) -> q k c", k=KT)
                if t % 2 == 0:
                    nc.scalar.activation(dst, src, AF.Copy, scale=FP8_SCALE)
                else:
                    nc.vector.tensor_scalar_mul(dst, src, FP8_SCALE)

            def emit_strip(r, csT, quad, cs_ps, first, last):
                """One sim strip at block-row r (global), local column
                strip csT of quadrant quad. A/C strips containing the
                diagonal are narrowed to skip fully-below-diagonal blocks."""
                lhsT = fT1 if r < PT else fT2
                rT = r % PT
                rhsT = fT1 if quad == "A" else fT2
                ko = 0                            # leading blocks skipped
                if quad != "B" and csT * 512 - rT * 128 < 128:
                    ko = rT - 4 * csT
                nw = 512 - 128 * ko               # strip width
                c0 = csT * 512 + ko * 128
                ps = mmp.tile([128, 512], F32, tag="ps",
                              name=f"ps{quad}_{csT}_{r}")
                for g in range(KT // 2):
                    nc.tensor.matmul(
                        ps[:, 0:nw],
                        lhsT[:, 2 * g:2 * g + 2, rT * 128:(rT + 1) * 128],
                        rhsT[:, 2 * g:2 * g + 2, c0:c0 + nw],
                        perf_mode=DR,
                        start=(g == 0), stop=(g == KT // 2 - 1))
                # previous strip's colsum lands here: by now its exp result
                # is ready, so the PE doesn't stall on it
                flush_pending()
                es = esp.tile([128, 512], BF16, tag="es",
                              name=f"es{quad}_{csT}_{r}")
                cs_g = csT if quad == "A" else half + csT
                if quad == "B":
                    nc.scalar.activation(es[:], ps[:], AF.Exp,
                                         scale=exp_scale,
                                         accum_out=acc[:, r, cs_g:cs_g + 1])
                    if csT * 4 <= rT <= csT * 4 + 3:
                        # pos_sim lives on this strip's diagonal block:
                        # extract it exactly from the f32 PSUM sims
                        jb = rT - csT * 4
                        sct = ttp.tile([128, 128], F32, tag="sct",
                                       name=f"sct_{r}")
                        nc.vector.scalar_tensor_tensor(
                            out=sct[:],
                            in0=ps[:, jb * 128:(jb + 1) * 128],
                            scalar=pos_scale,
                            in1=identB[:],
                            op0=ALU.mult,
                            op1=ALU.mult,
                            accum_out=poss20[:, rT:rT + 1])
                elif ko == 0 and csT * 512 - rT * 128 >= 128:
                    # strictly above the diagonal: plain exp + row sums
                    nc.scalar.activation(
                        es[:], ps[:], AF.Exp, scale=exp_scale,
                        accum_out=acc[:, r, cs_g:cs_g + 1])
                else:
                    # first block is the diagonal one: strict upper keep
                    nc.scalar.activation(es[:, 0:nw], ps[:, 0:nw], AF.Exp,
                                         scale=exp_scale)
                    # keep col > row: -1 + (-1)*p + 1*c >= 0
                    nc.gpsimd.affine_select(
                        out=es[:, 0:nw], in_=es[:, 0:nw],
                        compare_op=ALU.is_ge,
                        fill=0.0, base=-1, pattern=[[1, nw]],
                        channel_multiplier=-1)
                    nc.vector.tensor_reduce(acc[:, r, cs_g:cs_g + 1],
                                            es[:, 0:nw],
                                            axis=mybir.AxisListType.X,
                                            op=ALU.add)

                def colsum(es=es, first=first, last=last, vec=cs_ps,
                           ko=ko, nw=nw):
                    nc.tensor.matmul(vec[0:1, ko * 128:ko * 128 + nw],
                                     ones_bf[:], es[:, 0:nw],
                                     start=first, stop=last,
                                     skip_group_check=True)
                pending.append(colsum)

            def emit_mirror(base_chunk, nchunks):
                # transpose csum_sb chunks to token-major via K=1 matmuls
                for c in range(base_chunk, base_chunk + nchunks):
                    nc.tensor.matmul(cng[:, c:c + 1],
                                     csum_sb[0:1, c * 128:(c + 1) * 128],
                                     one_f32[:], start=True, stop=True)

            def run_view(view, src_dram, fT, tps, ps_vecs, strips_for_cs):
                """Emit the view's PT tiles interleaved with its strips.
                strips_for_cs(csT) -> ordered [(r, quad), ...]; the group
                for column strip csT becomes ready once tile 4*csT+3 is
                emitted. Ready strips are spread across the remaining tile
                slots so the PE stays fed while tiles DMA/normalize."""
                queue = []          # ready strips, annotated first/last
                for t in range(PT):
                    emit_tile(view, t, src_dram, fT, tps)
                    flush_pending()
                    if t % 4 == 3:
                        csT = t // 4
                        group = strips_for_cs(csT)
                        for i, (r, quad) in enumerate(group):
                            queue.append((r, csT, quad,
                                          i == 0, i == len(group) - 1))
                    tiles_left = PT - 1 - t
                    if tiles_left > 0 and queue:
                        n_emit = -(-len(queue) // (tiles_left + 1))
                        for _ in range(n_emit):
                            r, csT, quad, first, last = queue.pop(0)
                            emit_strip(r, csT, quad, ps_vecs[csT],
                                       first, last)
                for r, csT, quad, first, last in queue:
                    emit_strip(r, csT, quad, ps_vecs[csT], first, last)
                flush_pending()

            # ---- phase A: view 1 tiles + A-quadrant strips ----
            with tc.tile_pool(name="tpA_ps", bufs=2, space="PSUM") as tps, \
                 tc.tile_pool(name="psA", bufs=1, space="PSUM") as psAp:
                psA = [psAp.tile([1, 512], F32, name=f"psA{c}")
                       for c in range(half)]

                def a_strips(csT):
                    return [(r, "A") for r in range(min(4 * csT + 4, PT))]

                run_view(1, h1, fT1, tps, psA, a_strips)
                for c in range(half):
                    nc.vector.tensor_copy(
                        csum_sb[0:1, c * 512:(c + 1) * 512], psA[c][:])
            emit_mirror(0, PT)

            # ---- phase B: view 2 tiles + B/C strips ----
            with tc.tile_pool(name="tpB_ps", bufs=2, space="PSUM") as tps, \
                 tc.tile_pool(name="psB", bufs=1, space="PSUM") as psBp:
                psB = [psBp.tile([1, 512], F32, name=f"psB{c}")
                       for c in range(half)]

                def bc_strips(csT):
                    # C diag strips first (their exp->affine->reduce chain
                    # overlaps later strips; the emission leader is full
                    # width, as the PSUM colsum accumulation group needs),
                    # pure-upper C next, B strips last so the drain tail
                    # only needs cheap exp+accum
                    crows = list(range(min(4 * csT + 4, PT)))
                    diag = [(PT + rc, "C") for rc in crows
                            if csT * 512 - rc * 128 < 128]
                    pure = [(PT + rc, "C") for rc in crows
                            if csT * 512 - rc * 128 >= 128]
                    return diag + pure + [(r, "B") for r in range(PT)]

                run_view(2, h2, fT2, tps, psB, bc_strips)
                for c in range(half):
                    nc.vector.tensor_copy(
                        csum_sb[0:1, p + c * 512:p + (c + 1) * 512],
                        psB[c][:])
            emit_mirror(PT, PT)

            # ---- epilogue: final reduction chain ----
            with tc.tile_pool(name="ep", bufs=1) as ep, \
                 tc.tile_pool(name="ep_ps", bufs=1, space="PSUM") as epp:
                cngs = ep.tile([128, NB2], F32)
                nc.vector.tensor_copy(cngs[:], cng[:])
                ng = ep.tile([128, NB2], F32)
                nc.vector.tensor_reduce(ng[:], acc[:],
                                        axis=mybir.AxisListType.X,
                                        op=ALU.add)
                den = ep.tile([128, NB2], F32)
                nc.vector.tensor_add(den[:], ng[:], cngs[:])
                nc.vector.tensor_scalar_add(den[:], den[:], negK0[:])
                lg = ep.tile([128, NB2], F32)
                nc.scalar.activation(lg[:], den[:], AF.Ln)
                pm = ep.tile([128, NB2], F32)
                nc.vector.tensor_copy(pm[:, 0:PT], poss20[:])
                nc.vector.tensor_copy(pm[:, PT:NB2], poss20[:])
                d1 = ep.tile([128, NB2], F32)
                nc.vector.tensor_sub(d1[:], lg[:], pm[:])
                ptok = ep.tile([128, NB2], F32)
                tsum = ep.tile([128, 1], F32)
                nc.vector.scalar_tensor_tensor(
                    out=ptok[:], in0=d1[:], scalar=1.0, in1=msk24[:],
                    op0=ALU.mult, op1=ALU.mult, accum_out=tsum[:])
                lps = epp.tile([1, 1], F32)
                nc.tensor.matmul(lps[:], ones_col[:], tsum[:], start=True,
                                 stop=True)
                lsb = ep.tile([1, 1], F32)
                nc.vector.tensor_mul(lsb[:], lps[:], recn[:])
                nc.sync.dma_start(out[:], lsb[:])

    return nc


_NC = {}


def _get_nc(p: int) -> bass.Bass:
    if p not in _NC:
        _NC[p] = _build(p)
    return _NC[p]


def _mask_layout(mask_col: np.ndarray, p: int) -> np.ndarray:
    # token t = 128 * col + row  ->  [128, PT]
    return np.ascontiguousarray(
        mask_col.astype(np.float32).reshape(p // 128, 128).T)


def _in_maps(h1, h2, mask, p):
    maps = []
    for b in range(NCORES):
        idx = np.argsort(~mask[b], kind="stable")[:p]
        maps.append({
            "h1": np.ascontiguousarray(h1[b][idx]),
            "h2": np.ascontiguousarray(h2[b][idx]),
            "maskT": _mask_layout(mask[b][idx], p),
        })
    return maps


def kernel(last_hidden_states_1, last_hidden_states_2, token_mask_batch):
    h1 = np.ascontiguousarray(np.asarray(last_hidden_states_1,
                                         dtype=np.float32))
    h2 = np.ascontiguousarray(np.asarray(last_hidden_states_2,
                                         dtype=np.float32))
    mask = np.asarray(token_mask_batch).astype(bool)
    assert h1.shape == (NCORES, S, D), h1.shape

    p = P_MAIN if int(mask.sum(axis=1).max()) <= P_MAIN else S
    nc = _get_nc(p)
    res = run_bass_kernel_spmd(nc, _in_maps(h1, h2, mask, p),
                               list(range(NCORES)))
    vals = [np.asarray(res.results[b]["loss"], dtype=np.float32).reshape(())
            for b in range(NCORES)]
    return np.float32(np.mean(vals))


# revision 16
# speedup vs baseline: 2.2522x; 1.1907x over previous
"""ContraCLM token-level contrastive loss on 8 Trainium2 NeuronCores.

Data-parallel over the batch: core b handles sample b (B=8). Host-side,
each sample's unmasked tokens are compacted to the front (a pure gather /
layout transform; the kernel still sees real data rows for pads so norms
never hit 0/0) and padded to P=1024 (n ~ Binomial(1536, .5) ~ 768; the
build is generic in P with a P=1536 fallback if some n > 1024).

Per core, with P=1024, D=1024, T=0.05:

  f_v = l2norm(h_v) with pad rows zeroed (mask folded into the reciprocal
  norm scale); G_v = (8*f_v)^T stored [D, P] in fp8e4 (x8 keeps entries
  in e4m3's normal range).

  The 2P x 2P similarity matrix [[A B];[B^T C]] (A = f1 f1^T etc.) is
  symmetric, so only A/C upper-triangle strips and all of B are computed
  as [128, 512] PSUM strips (fp8 DoubleRow, K=1024). exp(sim/T) row sums
  come from the ScalarE activation free-dim accumulator; strips containing
  the diagonal get a strict-upper affine_select then a DVE row-sum. The
  mirrored (lower-triangle) contributions are recovered from column sums:
  a ones-vector stationary matmul streams each es strip into a per-column
  [1, 512] PSUM accumulator, which is transposed to token-major layout at
  the end via K=1 outer-product matmuls.

  B's diagonal is exp(pos_sim/T): it is left inside the row/col sums
  (denominator = Ng + pos exactly), and 20*pos_sim for the numerator is
  extracted exactly from the f32 PSUM sim diagonal with a fused
  tensor_tensor_reduce against an identity tile.

  Pad columns contribute exp(0)=1 to every row sum: subtract
  K0 = 2P - 2n. per_tok = ln(denom) - 20*pos_sim, masked mean over 2n
  tokens; per-sample means are averaged on the host (no collective).
"""

import sys

for _p in ("/opt/trn_rl_repo", "/opt/pypackages"):
    if _p not in sys.path:
        sys.path.append(_p)

from contextlib import ExitStack

import numpy as np

import bass_rust

import concourse.bass as bass
import concourse.tile as tile
from concourse import mybir
from concourse.bass_utils import run_bass_kernel_spmd
from concourse.masks import make_identity
from concourse.vector_clock import ScopedClock

# The walrus build in this container encodes at most 2 sync waits per
# instruction (bass_rust's inst_waits_full agrees), but Tile's semaphore
# assignment can attach more. Hoist excess waits onto unfusable same-engine
# NoOps immediately before the instruction — the engine executes its queue
# in order, so semantics are preserved.
_MAX_WAITS = 1


def _split_excess_waits(nc, ordered):
    for bb_name, insts in ordered.items():
        out = []
        changed = False
        for inst in insts:
            si = getattr(inst, "sync_info", None)
            waits = list(si.on_wait) if si is not None else []
            if len(waits) > _MAX_WAITS:
                changed = True
                extra, keep = waits[:-_MAX_WAITS], waits[-_MAX_WAITS:]
                for i in range(0, len(extra), _MAX_WAITS):
                    out.append(mybir.InstNoOp(
                        name=nc.get_next_instruction_name(),
                        sync_info=mybir.SyncInfo(
                            on_wait=extra[i:i + _MAX_WAITS], on_update=[]),
                        bass_nofuse=True,
                        engine=inst.engine,
                    ))
                si.on_wait = keep
            out.append(inst)
        if changed:
            insts[:] = out


_orig_lower_ordered_insts = tile.TileContext._lower_ordered_insts


def _patched_lower_ordered_insts(self, ordered):
    _split_excess_waits(self.nc, ordered)
    return _orig_lower_ordered_insts(self, ordered)


tile.TileContext._lower_ordered_insts = _patched_lower_ordered_insts


def _split_waits_drain_and_barrier(self, tick_clock, wait_clock):
    nc = self.nc
    probe = nc.sync.nop(nofuse=True)
    wait_clock.add_sem_waits(
        probe.ins, ScopedClock({None: tick_clock.global_clock}))
    si = probe.ins.sync_info
    waits = list(si.on_wait) if si is not None else []
    if len(waits) > _MAX_WAITS:
        si.on_wait = waits[:_MAX_WAITS]
        for i in range(_MAX_WAITS, len(waits), _MAX_WAITS):
            nxt = nc.sync.nop(nofuse=True)
            nxt.ins.sync_info = bass_rust.SyncInfo(
                on_wait=waits[i:i + _MAX_WAITS], on_update=[])
    nc.sync.drain()
    nc.all_engine_barrier()
    assert self.sems is not None
    popped = nc._tile_sem_poison_stack.pop()
    assert popped is self._sem_poison
    nc.clear_and_free_semaphores(list(self.sems.allocated().values()))
    nc.all_engine_barrier()


tile.TileContext._drain_and_barrier = _split_waits_drain_and_barrier

S, D, NCORES = 1536, 1024, 8
P_MAIN = 1024            # compacted+padded tokens per view
KT = D // 128            # 8 contraction tiles
TEMP_INV = 20.0          # 1 / 0.05
FP8_SCALE = 8.0          # f entries ~N(0, 1/32); x8 keeps them in e4m3's
                         # normal range (|f|*8 <~ 2, well under 240)
F32 = mybir.dt.float32
BF16 = mybir.dt.bfloat16
FP8 = mybir.dt.float8e4
AF = mybir.ActivationFunctionType
ALU = mybir.AluOpType
DR = mybir.MatmulPerfMode.DoubleRow


def _build(p: int, num_devices: int = NCORES) -> bass.Bass:
    PT = p // 128        # token tiles per view
    NB2 = 2 * PT         # block rows of the 2P x 2P matrix
    half = p // 512      # column strips per view
    exp_scale = TEMP_INV / (FP8_SCALE * FP8_SCALE)
    pos_scale = TEMP_INV / (FP8_SCALE * FP8_SCALE)

    nc = bass.Bass(num_devices=num_devices)
    h1 = nc.dram_tensor("h1", [p, D], F32, kind="ExternalInput")
    h2 = nc.dram_tensor("h2", [p, D], F32, kind="ExternalInput")
    # mask, pre-laid-out host-side as [128, PT] so token t = 128*col + row
    maskT = nc.dram_tensor("maskT", [128, PT], F32, kind="ExternalInput")
    out = nc.dram_tensor("loss", [1, 1], F32, kind="ExternalOutput")

    with tile.TileContext(nc) as tc, ExitStack() as ctx:
        const_pool = ctx.enter_context(tc.tile_pool(name="const", bufs=1))
        big = ctx.enter_context(tc.tile_pool(name="big", bufs=1))
        stat = ctx.enter_context(tc.tile_pool(name="stat", bufs=1))

        identB = const_pool.tile([128, 128], BF16)
        make_identity(nc, identB[:])
        ones_col = const_pool.tile([128, 1], F32)
        nc.gpsimd.memset(ones_col[:], 1.0)
        ones_sq = const_pool.tile([128, 128], F32)
        nc.gpsimd.memset(ones_sq[:], 1.0)
        ones_bf = const_pool.tile([128, 1], BF16)
        nc.gpsimd.memset(ones_bf[:], 1.0)
        one_f32 = const_pool.tile([1, 1], F32)
        nc.gpsimd.memset(one_f32[:], 1.0)
        msk = const_pool.tile([128, PT], F32)
        nc.sync.dma_start(msk[:], maskT[:])

        # tile-major transposed features: fT[:, t, k*128+c] holds
        # (8*f)^T[d = k*128 + partition, token = t*128 + c] in fp8e4, so
        # each token tile's transpose lands as one contiguous copy
        fT1 = big.tile([128, PT, KT * 128], FP8)
        fT2 = big.tile([128, PT, KT * 128], FP8)
        acc = stat.tile([128, NB2, 2 * half], F32)   # per-strip row sums
        poss20 = stat.tile([128, PT], F32)       # 20 * pos_sim
        csum_sb = stat.tile([1, 2 * p], F32)     # mirror column sums
        msk24 = stat.tile([128, NB2], F32)
        negK0 = stat.tile([128, 1], F32)
        recn = stat.tile([1, 1], F32)

        # zero row-sum slots never written (below-diagonal A/C strips)
        nc.gpsimd.memset(acc[:], 0.0)

        # ---- HAM warmup: keep the PE busy from t~0 so it upclocks ----
        with tc.tile_pool(name="warm", bufs=1, space="PSUM") as wp:
            wps = wp.tile([128, 128], F32)
            for _ in range(24):
                nc.tensor.matmul(wps[:], identB[:], identB[:],
                                 start=True, stop=True)

        # ---- phase 0: mask-only precomputes ----
        with tc.tile_pool(name="ep0", bufs=1) as ep0, \
             tc.tile_pool(name="ep0_ps", bufs=1, space="PSUM") as ep0p:
            msum = ep0.tile([128, 1], F32)
            nc.vector.tensor_reduce(msum[:], msk[:],
                                    axis=mybir.AxisListType.X, op=ALU.add)
            nps = ep0p.tile([128, 1], F32)
            nc.tensor.matmul(nps[:], ones_sq[:], msum[:], start=True,
                             stop=True)
            # -K0 = 2n - 2P, broadcast to all partitions
            nc.scalar.activation(negK0[:], nps[:], AF.Copy, scale=2.0,
                                 bias=float(-2 * p))
            n2c = ep0.tile([1, 1], F32)
            nc.scalar.activation(n2c[:], nps[0:1, :], AF.Copy, scale=2.0)
            nc.vector.reciprocal(recn[:], n2c[:])   # 1/(2n)
            nc.vector.tensor_copy(msk24[:, 0:PT], msk[:])
            nc.vector.tensor_copy(msk24[:, PT:NB2], msk[:])

        with tc.tile_pool(name="mm_ps", bufs=3, space="PSUM") as mmp, \
             tc.tile_pool(name="cng_ps", bufs=1, space="PSUM") as cngp, \
             tc.tile_pool(name="es", bufs=4) as esp, \
             tc.tile_pool(name="ht", bufs=3) as htp, \
             tc.tile_pool(name="scr", bufs=2) as scr, \
             tc.tile_pool(name="sc", bufs=4) as scp, \
             tc.tile_pool(name="tt", bufs=2) as ttp:

            cng = cngp.tile([128, NB2], F32)     # mirror sums, token-major
            pending = []                         # deferred colsum matmuls

            def flush_pending():
                while pending:
                    pending.pop(0)()

            def emit_tile(v, t, src_dram, fT, tps):
                ht = htp.tile([128, D], F32, tag="ht", name=f"ht{v}_{t}")
                nc.sync.dma_start(ht[:], src_dram[t * 128:(t + 1) * 128, :])
                sq = scr.tile([128, D], BF16, tag="sq", name=f"sq{v}_{t}")
                ss = scp.tile([128, 1], F32, tag="ss", name=f"ss{v}_{t}")
                # squared-norm accumulate on DVE, keeping ScalarE free for
                # exps (and avoiding its activation-table thrash)
                nc.vector.scalar_tensor_tensor(
                    out=sq[:], in0=ht[:], scalar=1.0, in1=ht[:],
                    op0=ALU.mult, op1=ALU.mult, accum_out=ss[:])
                # 1/sqrt(ss) = exp(-0.5*ln(ss)): Ln/Exp share a ScalarE
                # table with the strip exps, so no ACT_TABLE_LOAD thrash
                lnss = scp.tile([128, 1], F32, tag="ln", name=f"ln{v}_{t}")
                nc.scalar.activation(lnss[:], ss[:], AF.Ln)
                sc = scp.tile([128, 1], F32, tag="sc", name=f"sc{v}_{t}")
                nc.scalar.activation(sc[:], lnss[:], AF.Exp, scale=-0.5)
                scm = scp.tile([128, 1], F32, tag="scm", name=f"scm{v}_{t}")
                nc.vector.tensor_mul(scm[:], sc[:], msk[:, t:t + 1])
                fnb = scr.tile([128, D], BF16, tag="fn", name=f"fn{v}_{t}")
                nc.vector.tensor_scalar_mul(fnb[:], ht[:], scm[:])
                pt = tps.tile([128, D], BF16, tag="pt", name=f"pt{v}_{t}")
                for k in range(KT):
                    nc.tensor.transpose(pt[:, k * 128:(k + 1) * 128],
                                        fnb[:, k * 128:(k + 1) * 128],
                                        identB[:])
                # quantize to fp8 (x8) while moving PSUM->SBUF; the
                # tile-major fT makes this a contiguous copy
                nc.scalar.activation(fT[:, t, :], pt[:], AF.Copy,
                                     scale=FP8_SCALE)

            def emit_strip(r, csT, quad, cs_ps, first, last):
                """One sim strip at block-row r (global), local column
                strip csT of quadrant quad. A/C strips containing the
                diagonal are narrowed to skip fully-below-diagonal blocks."""
                lhsT = fT1 if r < PT else fT2
                rT = r % PT
                rhsT = fT1 if quad == "A" else fT2
                ko = 0                            # leading blocks skipped
                if quad != "B" and csT * 512 - rT * 128 < 128:
                    ko = rT - 4 * csT
                nw = 512 - 128 * ko               # strip width
                ps = mmp.tile([128, 512], F32, tag="ps",
                              name=f"ps{quad}_{csT}_{r}")
                rhs4 = rhsT[:, 4 * csT + ko:4 * csT + 4, :].rearrange(
                    "q t (k c) -> q k t c", k=KT)
                lhs3 = lhsT[:, rT, :].rearrange("q (k c) -> q k c", k=KT)
                for g in range(KT // 2):
                    nc.tensor.matmul(
                        ps[:, 0:nw],
                        lhs3[:, 2 * g:2 * g + 2, :],
                        rhs4[:, 2 * g:2 * g + 2, :, :],
                        perf_mode=DR,
                        start=(g == 0), stop=(g == KT // 2 - 1))
                # previous strip's colsum lands here: by now its exp result
                # is ready, so the PE doesn't stall on it
                flush_pending()
                es = esp.tile([128, 512], BF16, tag="es",
                              name=f"es{quad}_{csT}_{r}")
                cs_g = csT if quad == "A" else half + csT
                if quad == "B":
                    nc.scalar.activation(es[:], ps[:], AF.Exp,
                                         scale=exp_scale,
                                         accum_out=acc[:, r, cs_g:cs_g + 1])
                    if csT * 4 <= rT <= csT * 4 + 3:
                        # pos_sim lives on this strip's diagonal block:
                        # extract it exactly from the f32 PSUM sims
                        jb = rT - csT * 4
                        sct = ttp.tile([128, 128], F32, tag="sct",
                                       name=f"sct_{r}")
                        nc.vector.scalar_tensor_tensor(
                            out=sct[:],
                            in0=ps[:, jb * 128:(jb + 1) * 128],
                            scalar=pos_scale,
                            in1=identB[:],
                            op0=ALU.mult,
                            op1=ALU.mult,
                            accum_out=poss20[:, rT:rT + 1])
                elif ko == 0 and csT * 512 - rT * 128 >= 128:
                    # strictly above the diagonal: plain exp + row sums
                    nc.scalar.activation(
                        es[:], ps[:], AF.Exp, scale=exp_scale,
                        accum_out=acc[:, r, cs_g:cs_g + 1])
                else:
                    # first block is the diagonal one: strict upper keep
                    nc.scalar.activation(es[:, 0:nw], ps[:, 0:nw], AF.Exp,
                                         scale=exp_scale)
                    # keep col > row: -1 + (-1)*p + 1*c >= 0
                    nc.gpsimd.affine_select(
                        out=es[:, 0:nw], in_=es[:, 0:nw],
                        compare_op=ALU.is_ge,
                        fill=0.0, base=-1, pattern=[[1, nw]],
                        channel_multiplier=-1)
                    nc.vector.tensor_reduce(acc[:, r, cs_g:cs_g + 1],
                                            es[:, 0:nw],
                                            axis=mybir.AxisListType.X,
                                            op=ALU.add)

                def colsum(es=es, first=first, last=last, vec=cs_ps,
                           ko=ko, nw=nw):
                    nc.tensor.matmul(vec[0:1, ko * 128:ko * 128 + nw],
                                     ones_bf[:], es[:, 0:nw],
                                     start=first, stop=last,
                                     skip_group_check=True)
                pending.append(colsum)

            def emit_mirror(base_chunk, nchunks):
                # transpose csum_sb chunks to token-major via K=1 matmuls
                for c in range(base_chunk, base_chunk + nchunks):
                    nc.tensor.matmul(cng[:, c:c + 1],
                                     csum_sb[0:1, c * 128:(c + 1) * 128],
                                     one_f32[:], start=True, stop=True)

            def run_view(view, src_dram, fT, tps, ps_vecs, strips_for_cs):
                """Emit the view's PT tiles interleaved with its strips.
                strips_for_cs(csT) -> ordered [(r, quad), ...]; the group
                for column strip csT becomes ready once tile 4*csT+3 is
                emitted. Ready strips are spread across the remaining tile
                slots so the PE stays fed while tiles DMA/normalize."""
                queue = []          # ready strips, annotated first/last
                for t in range(PT):
                    emit_tile(view, t, src_dram, fT, tps)
                    flush_pending()
                    if t % 4 == 3:
                        csT = t // 4
                        group = strips_for_cs(csT)
                        for i, (r, quad) in enumerate(group):
                            queue.append((r, csT, quad,
                                          i == 0, i == len(group) - 1))
                    tiles_left = PT - 1 - t
                    if tiles_left > 0 and queue:
                        n_emit = -(-len(queue) // (tiles_left + 1))
                        for _ in range(n_emit):
                            r, csT, quad, first, last = queue.pop(0)
                            emit_strip(r, csT, quad, ps_vecs[csT],
                                       first, last)
                for r, csT, quad, first, last in queue:
                    emit_strip(r, csT, quad, ps_vecs[csT], first, last)
                flush_pending()

            # ---- phase A: view 1 tiles + A-quadrant strips ----
            with tc.tile_pool(name="tpA_ps", bufs=2, space="PSUM") as tps, \
                 tc.tile_pool(name="psA", bufs=1, space="PSUM") as psAp:
                psA = [psAp.tile([1, 512], F32, name=f"psA{c}")
                       for c in range(half)]

                def a_strips(csT):
                    return [(r, "A") for r in range(min(4 * csT + 4, PT))]

                run_view(1, h1, fT1, tps, psA, a_strips)
                for c in range(half):
                    nc.vector.tensor_copy(
                        csum_sb[0:1, c * 512:(c + 1) * 512], psA[c][:])
            emit_mirror(0, PT)

            # ---- phase B: view 2 tiles + B/C strips ----
            with tc.tile_pool(name="tpB_ps", bufs=2, space="PSUM") as tps, \
                 tc.tile_pool(name="psB", bufs=1, space="PSUM") as psBp:
                psB = [psBp.tile([1, 512], F32, name=f"psB{c}")
                       for c in range(half)]

                def bc_strips(csT):
                    # C diag strips first (their exp->affine->reduce chain
                    # overlaps later strips; the emission leader is full
                    # width, as the PSUM colsum accumulation group needs),
                    # pure-upper C next, B strips last so the drain tail
                    # only needs cheap exp+accum
                    crows = list(range(min(4 * csT + 4, PT)))
                    diag = [(PT + rc, "C") for rc in crows
                            if csT * 512 - rc * 128 < 128]
                    pure = [(PT + rc, "C") for rc in crows
                            if csT * 512 - rc * 128 >= 128]
                    return diag + pure + [(r, "B") for r in range(PT)]

                run_view(2, h2, fT2, tps, psB, bc_strips)
                for c in range(half):
                    nc.vector.tensor_copy(
                        csum_sb[0:1, p + c * 512:p + (c + 1) * 512],
                        psB[c][:])
            emit_mirror(PT, PT)

            # ---- epilogue: final reduction chain ----
            with tc.tile_pool(name="ep", bufs=1) as ep, \
                 tc.tile_pool(name="ep_ps", bufs=1, space="PSUM") as epp:
                cngs = ep.tile([128, NB2], F32)
                nc.vector.tensor_copy(cngs[:], cng[:])
                ng = ep.tile([128, NB2], F32)
                nc.vector.tensor_reduce(ng[:], acc[:],
                                        axis=mybir.AxisListType.X,
                                        op=ALU.add)
                den = ep.tile([128, NB2], F32)
                nc.vector.tensor_add(den[:], ng[:], cngs[:])
                nc.vector.tensor_scalar_add(den[:], den[:], negK0[:])
                lg = ep.tile([128, NB2], F32)
                nc.scalar.activation(lg[:], den[:], AF.Ln)
                pm = ep.tile([128, NB2], F32)
                nc.vector.tensor_copy(pm[:, 0:PT], poss20[:])
                nc.vector.tensor_copy(pm[:, PT:NB2], poss20[:])
                d1 = ep.tile([128, NB2], F32)
                nc.vector.tensor_sub(d1[:], lg[:], pm[:])
                ptok = ep.tile([128, NB2], F32)
                tsum = ep.tile([128, 1], F32)
                nc.vector.scalar_tensor_tensor(
                    out=ptok[:], in0=d1[:], scalar=1.0, in1=msk24[:],
                    op0=ALU.mult, op1=ALU.mult, accum_out=tsum[:])
                lps = epp.tile([1, 1], F32)
                nc.tensor.matmul(lps[:], ones_col[:], tsum[:], start=True,
                                 stop=True)
                lsb = ep.tile([1, 1], F32)
                nc.vector.tensor_mul(lsb[:], lps[:], recn[:])
                nc.sync.dma_start(out[:], lsb[:])

    return nc


_NC = {}


def _get_nc(p: int) -> bass.Bass:
    if p not in _NC:
        _NC[p] = _build(p)
    return _NC[p]


def _mask_layout(mask_col: np.ndarray, p: int) -> np.ndarray:
    # token t = 128 * col + row  ->  [128, PT]
    return np.ascontiguousarray(
        mask_col.astype(np.float32).reshape(p // 128, 128).T)


def _in_maps(h1, h2, mask, p):
    maps = []
    for b in range(NCORES):
        idx = np.argsort(~mask[b], kind="stable")[:p]
        maps.append({
            "h1": np.ascontiguousarray(h1[b][idx]),
            "h2": np.ascontiguousarray(h2[b][idx]),
            "maskT": _mask_layout(mask[b][idx], p),
        })
    return maps


def kernel(last_hidden_states_1, last_hidden_states_2, token_mask_batch):
    h1 = np.ascontiguousarray(np.asarray(last_hidden_states_1,
                                         dtype=np.float32))
    h2 = np.ascontiguousarray(np.asarray(last_hidden_states_2,
                                         dtype=np.float32))
    mask = np.asarray(token_mask_batch).astype(bool)
    assert h1.shape == (NCORES, S, D), h1.shape

    p = P_MAIN if int(mask.sum(axis=1).max()) <= P_MAIN else S
    nc = _get_nc(p)
    res = run_bass_kernel_spmd(nc, _in_maps(h1, h2, mask, p),
                               list(range(NCORES)))
    vals = [np.asarray(res.results[b]["loss"], dtype=np.float32).reshape(())
            for b in range(NCORES)]
    return np.float32(np.mean(vals))
